# revision 9
# baseline (speedup 1.0000x reference)
"""Trainium2 Bass kernel for nn_BertSelfAttention_257698038467.

Reference computation (bs=4, s=2048, d=256, h=4):
    Q = (x @ Wq).view(bs, h, s, d)      # raw C-order reinterpretation!
    K = (x @ Wk).view(bs, h, s, d)
    probs = softmax(Q @ K^T / 16)
    out   = einsum('bhij,bjd->bhid', probs, x)   # value = raw x, shared
    out   = out.transpose(0,2,1,3).reshape(bs,s,h*d) @ Wv + x
    out   = layernorm(out, gamma, beta)
    return out, probs

The C-order view means, per batch b and head hh (with local query index
i' = 4r + a, a = i' % 4):
    Q[b,hh,i',:] = x[b, hh*512 + r_q, :] @ Wq[:, a*256:(a+1)*256]
    K[b,hh,j ,:] = x[b, hh*512 + r_k, :] @ Wk[:, b2*256:(b2+1)*256],  j = 4*r_k + b2

Sharding: 8 cores = (4 batches) x (2 query-halves of 1024 rows).  Each core
computes all 4 heads for its query rows: scores, softmax, probs output, PV
with the shared x as value, the Wv projection, residual and layernorm.  No
collectives are needed; the host concatenates per-core outputs.

All matmuls run as float32-in-memory bitcast to float32r (full PE rate).
"""

import os
from contextlib import ExitStack

import numpy as np

import concourse.bass as bass
import concourse.tile as tile
from concourse import bacc, mybir
from concourse.bass_utils import run_bass_kernel_spmd
from concourse.masks import make_identity

F32 = mybir.dt.float32
F32R = mybir.dt.float32r
LN_EPS = 1e-5
INV_SQRT_D = 1.0 / 16.0  # 1/sqrt(256)

N_CORES = 8
S = 2048          # full sequence
D = 256           # model dim
H = 4             # heads
SQ = 1024         # query rows per core
N_IC = SQ // 128  # 8 i'-chunks per head


def build_nc():
    nc = bacc.Bacc(
        "TRN2",
        target_bir_lowering=False,
        debug=False,
        enable_asserts=True,
        num_devices=N_CORES,
    )

    # ---- DRAM I/O ----------------------------------------------------------
    xb_d = nc.dram_tensor("xb", [S, D], F32, kind="ExternalInput")
    xbT_d = nc.dram_tensor("xbT", [D, S], F32, kind="ExternalInput")
    xqT_d = nc.dram_tensor("xqT", [H, D, 256], F32, kind="ExternalInput")
    xres_d = nc.dram_tensor("xres", [SQ, D], F32, kind="ExternalInput")
    wq_d = nc.dram_tensor("wq", [D, H * D], F32, kind="ExternalInput")
    wk_d = nc.dram_tensor("wk", [D, H * D], F32, kind="ExternalInput")
    wv_d = nc.dram_tensor("wv", [H * D, D], F32, kind="ExternalInput")
    gamma_d = nc.dram_tensor("gamma", [D], F32, kind="ExternalInput")
    beta_d = nc.dram_tensor("beta", [D], F32, kind="ExternalInput")
    out_d = nc.dram_tensor("out", [SQ, D], F32, kind="ExternalOutput")
    probs_d = nc.dram_tensor("probs", [H, SQ, S], F32, kind="ExternalOutput")

    with tile.TileContext(nc) as tc, ExitStack() as ctx:
        # ---- pools ---------------------------------------------------------
        singles = ctx.enter_context(tc.tile_pool(name="singles", bufs=1))
        xt_pool = ctx.enter_context(tc.tile_pool(name="xt", bufs=2))
        kt_pool = ctx.enter_context(tc.tile_pool(name="kt", bufs=1))
        es_pool = ctx.enter_context(tc.tile_pool(name="es", bufs=2))
        pr_pool = ctx.enter_context(tc.tile_pool(name="pr", bufs=2))
        est_pool = ctx.enter_context(tc.tile_pool(name="est", bufs=2))
        sm_pool = ctx.enter_context(tc.tile_pool(name="sm", bufs=4))
        o_pool = ctx.enter_context(tc.tile_pool(name="o", bufs=3))
        ln_pool = ctx.enter_context(tc.tile_pool(name="ln", bufs=4))

        # PSUM pools (8 banks of [128 x 2KB] total)
        sp_ps = ctx.enter_context(tc.tile_pool(name="sp", bufs=4, space="PSUM"))
        tp_ps = ctx.enter_context(tc.tile_pool(name="tp", bufs=2, space="PSUM"))
        at_ps = ctx.enter_context(tc.tile_pool(name="at", bufs=1, space="PSUM"))
        pj_ps = ctx.enter_context(tc.tile_pool(name="pj", bufs=1, space="PSUM"))

        # ---- resident tiles ------------------------------------------------
        identity_f = singles.tile([128, 128], F32)
        make_identity(nc, identity_f[:])
        identity = singles.tile([128, 128], F32R)
        nc.vector.tensor_copy(identity[:], identity_f[:])

        # x rows for PV: xb_sb[p, t, dd] = x[t*128+p, dd]
        xb_sb = singles.tile([128, 16, D], F32R)
        nc.sync.dma_start(
            xb_sb[:], xb_d.ap().rearrange("(t p) d -> p t d", p=128).bitcast(F32R)
        )

        # weights: w[c*128+p, n] -> [p, c, n]
        wq_sb = singles.tile([128, 2, H * D], F32R)
        nc.sync.dma_start(
            wq_sb[:], wq_d.ap().rearrange("(c p) n -> p c n", p=128).bitcast(F32R)
        )
        wk_sb = singles.tile([128, 2, H * D], F32R)
        nc.sync.dma_start(
            wk_sb[:], wk_d.ap().rearrange("(c p) n -> p c n", p=128).bitcast(F32R)
        )
        wv_sb = singles.tile([128, 8, D], F32R)
        nc.sync.dma_start(
            wv_sb[:], wv_d.ap().rearrange("(t p) d -> p t d", p=128).bitcast(F32R)
        )

        gamma_sb = singles.tile([128, D], F32)
        nc.gpsimd.dma_start(
            gamma_sb[:],
            bass.AP(tensor=gamma_d, offset=0, ap=[[0, 128], [1, D]]),
        )
        beta_sb = singles.tile([128, D], F32)
        nc.gpsimd.dma_start(
            beta_sb[:],
            bass.AP(tensor=beta_d, offset=0, ap=[[0, 128], [1, D]]),
        )
        eps_sb = singles.tile([128, 1], F32)
        nc.vector.memset(eps_sb[:], LN_EPS)

        # attT_sb[p, h, c2, i'] = att_h^T[c2*128+p, i']   (unnormalized-free:
        # probs used for PV are already normalized)
        attT_sb = singles.tile([128, H, 2, SQ], F32R)

        # ---- per-head attention -------------------------------------------
        for hh in range(H):
            # x^T columns for this head's K rows: xT_h[p, c, r] = x[h*512+r, c*128+p]
            xT_h = xt_pool.tile([128, 2, 512], F32R, tag="xT")
            nc.sync.dma_start(
                xT_h[:],
                xbT_d.ap()[:, hh * 512:(hh + 1) * 512].rearrange(
                    "(c p) j -> p c j", p=128
                ).bitcast(F32R),
            )
            # x^T columns for this head's Q rows
            xqT_h = xt_pool.tile([128, 2, 256], F32R, tag="xqT")
            nc.sync.dma_start(
                xqT_h[:],
                xqT_d.ap()[hh].rearrange("(c p) r -> p c r", p=128).bitcast(F32R),
            )

            # K^T_h[dd, j]: KT[p, c2, r, a] = K^T[c2*128+p, 4r+a]
            KT = kt_pool.tile([128, 2, 512, 4], F32R, tag="KT")
            QT = kt_pool.tile([128, 2, 256, 4], F32R, tag="QT")
            for a in range(4):
                for c2 in range(2):
                    pk = sp_ps.tile([128, 512], F32, tag="sps")
                    for c in range(2):
                        nc.tensor.matmul(
                            pk[:],
                            wk_sb[:, c, a * 256 + c2 * 128:a * 256 + c2 * 128 + 128],
                            xT_h[:, c, :],
                            start=(c == 0),
                            stop=(c == 1),
                        )
                    nc.vector.tensor_copy(KT[:, c2, :, a], pk[:])

                    pq = sp_ps.tile([128, 512], F32, tag="sps")
                    for c in range(2):
                        nc.tensor.matmul(
                            pq[:, 0:256],
                            wq_sb[:, c, a * 256 + c2 * 128:a * 256 + c2 * 128 + 128],
                            xqT_h[:, c, :],
                            start=(c == 0),
                            stop=(c == 1),
                        )
                    nc.vector.tensor_copy(QT[:, c2, :, a], pq[:, 0:256])

            # i'-chunks, processed in groups of 2 (PV free dim = 256)
            for icg in range(N_IC // 2):
                expST = est_pool.tile([128, 16, 2, 128], F32R, tag="expST")
                for ic2 in range(2):
                    ic = icg * 2 + ic2
                    # scores S[i', j] for i' chunk, all j, accumulated over dd
                    expS = es_pool.tile([128, S], F32, tag="expS")
                    sums4 = sm_pool.tile([128, 4], F32, tag="sums4")
                    for jc in range(4):
                        ps = sp_ps.tile([128, 512], F32, tag="sps")
                        for c2 in range(2):
                            nc.tensor.matmul(
                                ps[:],
                                QT[:, c2, ic * 32:(ic + 1) * 32, :],
                                KT[:, c2, jc * 128:(jc + 1) * 128, :],
                                start=(c2 == 0),
                                stop=(c2 == 1),
                            )
                        # exp(S/16), with per-row running sum
                        nc.scalar.activation(
                            out=expS[:, jc * 512:(jc + 1) * 512],
                            in_=ps[:],
                            func=mybir.ActivationFunctionType.Exp,
                            scale=INV_SQRT_D,
                            accum_out=sums4[:, jc:jc + 1],
                        )
                    sums1 = sm_pool.tile([128, 1], F32, tag="sums1")
                    nc.vector.tensor_reduce(
                        out=sums1[:],
                        in_=sums4[:],
                        axis=mybir.AxisListType.X,
                        op=mybir.AluOpType.add,
                    )
                    recip = sm_pool.tile([128, 1], F32, tag="recip")
                    nc.vector.reciprocal(recip[:], sums1[:])

                    # normalized probs (fp32) -> HBM
                    probs_t = pr_pool.tile([128, S], F32R, tag="probs")
                    nc.vector.tensor_scalar_mul(probs_t[:], expS[:], recip[:])
                    nc.sync.dma_start(
                        probs_d.ap()[hh, ic * 128:(ic + 1) * 128, :],
                        probs_t[:].bitcast(F32),
                    )

                    # bridge: transpose probs to [j, i'] layout for PV
                    for jg in range(4):
                        tp = tp_ps.tile([128, 4, 128], F32R, tag="tp")
                        for t4 in range(4):
                            jc2 = jg * 4 + t4
                            nc.tensor.transpose(
                                tp[:, t4, :],
                                probs_t[:, jc2 * 128:(jc2 + 1) * 128],
                                identity[:],
                            )
                        if jg % 2 == 0:
                            nc.scalar.copy(
                                expST[:, jg * 4:(jg + 1) * 4, ic2, :], tp[:]
                            )
                        else:
                            nc.vector.tensor_copy(
                                expST[:, jg * 4:(jg + 1) * 4, ic2, :], tp[:]
                            )

                # PV: attT[dd, i'] = sum_j x[j, dd] * P[i', j]
                for c2 in range(2):
                    pa = at_ps.tile([128, 256], F32, tag="at")
                    for jc2 in range(16):
                        nc.tensor.matmul(
                            pa[:],
                            xb_sb[:, jc2, c2 * 128:(c2 + 1) * 128],
                            expST[:, jc2, :, :],
                            start=(jc2 == 0),
                            stop=(jc2 == 15),
                        )
                    nc.vector.tensor_copy(
                        attT_sb[:, hh, c2, icg * 256:(icg + 1) * 256], pa[:]
                    )

        # ---- output projection + residual + layernorm ---------------------
        for ic in range(N_IC):
            pj = pj_ps.tile([128, 256], F32, tag="pj")
            k = 0
            for hh in range(H):
                for c2 in range(2):
                    nc.tensor.matmul(
                        pj[:],
                        attT_sb[:, hh, c2, ic * 128:(ic + 1) * 128],
                        wv_sb[:, hh * 2 + c2, :],
                        start=(k == 0),
                        stop=(k == 7),
                    )
                    k += 1
            xres_t = o_pool.tile([128, D], F32, tag="xres")
            nc.sync.dma_start(xres_t[:], xres_d.ap()[ic * 128:(ic + 1) * 128, :])
            res = o_pool.tile([128, D], F32, tag="res")
            nc.vector.tensor_add(res[:], pj[:], xres_t[:])

            st = ln_pool.tile([128, 6], F32, tag="st")
            nc.vector.bn_stats(st[:], res[:])
            mv = ln_pool.tile([128, 2], F32, tag="mv")
            nc.vector.bn_aggr(mv[:], st[:])
            sq = ln_pool.tile([128, 1], F32, tag="sq")
            nc.scalar.activation(
                out=sq[:],
                in_=mv[:, 1:2],
                func=mybir.ActivationFunctionType.Sqrt,
                bias=eps_sb[:],
            )
            rstd = ln_pool.tile([128, 1], F32, tag="rstd")
            nc.vector.reciprocal(rstd[:], sq[:])

            t1 = o_pool.tile([128, D], F32, tag="t1")
            nc.vector.tensor_scalar(
                out=t1[:],
                in0=res[:],
                scalar1=mv[:, 0:1],
                scalar2=rstd[:],
                op0=mybir.AluOpType.subtract,
                op1=mybir.AluOpType.mult,
            )
            t2 = o_pool.tile([128, D], F32, tag="t2")
            nc.vector.tensor_mul(t2[:], t1[:], gamma_sb[:])
            t3 = o_pool.tile([128, D], F32, tag="t3")
            nc.vector.tensor_add(t3[:], t2[:], beta_sb[:])
            nc.sync.dma_start(out_d.ap()[ic * 128:(ic + 1) * 128, :], t3[:])

    nc.compile()
    return nc


_NC = None
LAST_RESULTS = None


def _get_nc():
    global _NC
    if _NC is None:
        _NC = build_nc()
    return _NC


def make_in_maps(x, Wq, Wk, Wv, gamma, beta):
    x = np.ascontiguousarray(x, dtype=np.float32)
    Wq = np.ascontiguousarray(Wq, dtype=np.float32)
    Wk = np.ascontiguousarray(Wk, dtype=np.float32)
    Wv = np.ascontiguousarray(Wv, dtype=np.float32)
    gamma = np.ascontiguousarray(gamma, dtype=np.float32)
    beta = np.ascontiguousarray(beta, dtype=np.float32)
    in_maps = []
    for c in range(N_CORES):
        b, qh = divmod(c, 2)
        qoff = qh * SQ
        qoff4 = qoff // 4
        xb = np.ascontiguousarray(x[b])
        xbT = np.ascontiguousarray(xb.T)
        xqT = np.stack(
            [
                np.ascontiguousarray(xb[hh * 512 + qoff4:hh * 512 + qoff4 + 256].T)
                for hh in range(H)
            ]
        )
        xres = np.ascontiguousarray(xb[qoff:qoff + SQ])
        in_maps.append(
            {
                "xb": xb,
                "xbT": xbT,
                "xqT": xqT,
                "xres": xres,
                "wq": Wq,
                "wk": Wk,
                "wv": Wv,
                "gamma": gamma,
                "beta": beta,
            }
        )
    return in_maps


def kernel(x, Wq, Wk, Wv, gamma, beta):
    global LAST_RESULTS
    nc = _get_nc()
    in_maps = make_in_maps(x, Wq, Wk, Wv, gamma, beta)
    res = run_bass_kernel_spmd(
        nc,
        in_maps,
        core_ids=list(range(N_CORES)),
        trace=bool(int(os.environ.get("KERNEL_TRACE", "0"))),
    )
    LAST_RESULTS = res
    bs = x.shape[0]
    out = np.zeros((bs, S, D), np.float32)
    probs = np.zeros((bs, H, S, S), np.float32)
    for c in range(N_CORES):
        b, qh = divmod(c, 2)
        qoff = qh * SQ
        r = res.results[c]
        out[b, qoff:qoff + SQ] = r["out"]
        probs[b, :, qoff:qoff + SQ, :] = r["probs"]
    return out, probs


# revision 10
# speedup vs baseline: 349478.9877x; 349478.9877x over previous
"""Trainium2 Bass kernel for nn_BertSelfAttention_257698038467.

Reference computation (bs=4, s=2048, d=256, h=4):
    Q = (x @ Wq).view(bs, h, s, d)      # raw C-order reinterpretation!
    K = (x @ Wk).view(bs, h, s, d)
    probs = softmax(Q @ K^T / 16)
    out   = einsum('bhij,bjd->bhid', probs, x)   # value = raw x, shared
    out   = out.transpose(0,2,1,3).reshape(bs,s,h*d) @ Wv + x
    out   = layernorm(out, gamma, beta)
    return out, probs

The C-order view means, per batch b and head hh (with local query index
i' = 4r + a, a = i' % 4):
    Q[b,hh,i',:] = x[b, hh*512 + r_q, :] @ Wq[:, a*256:(a+1)*256]
    K[b,hh,j ,:] = x[b, hh*512 + r_k, :] @ Wk[:, b2*256:(b2+1)*256],  j = 4*r_k + b2

Sharding: 8 cores = (4 batches) x (2 query-halves of 1024 rows).  Each core
computes all 4 heads for its query rows: scores, softmax, probs output, PV
with the shared x as value, the Wv projection, residual and layernorm.  No
collectives are needed; the host concatenates per-core outputs.

All matmuls run as float32-in-memory bitcast to float32r (full PE rate).
"""

import os
from contextlib import ExitStack

import numpy as np

import concourse.bass as bass
import concourse.tile as tile
from concourse import bacc, mybir
from concourse.bass_utils import run_bass_kernel_spmd
from concourse.masks import make_identity

F32 = mybir.dt.float32
F32R = mybir.dt.float32r
BF16 = mybir.dt.bfloat16
LN_EPS = 1e-5
INV_SQRT_D = 1.0 / 16.0  # 1/sqrt(256)

N_CORES = 8
S = 2048          # full sequence
D = 256           # model dim
H = 4             # heads
SQ = 1024         # query rows per core
N_IC = SQ // 128  # 8 i'-chunks per head

# tunables (A/B'd via TimelineSim)
CFG = {
    "bridge_bf16": False,  # cast probs->bf16 (gpsimd) before transpose/PV
    "icg": 2,              # i'-chunks per PV group (PV free dim = icg*128)
    "kt_bufs": 1,
    "sp_bufs": 4,
    "tp_bufs": 2,
    "at_bufs": 1,
}


def build_nc(cfg=None):
    cfg = {**CFG, **(cfg or {})}
    bridge_bf16 = cfg["bridge_bf16"]
    icg_n = cfg["icg"]
    bridge_dt = BF16 if bridge_bf16 else F32R
    xb_dt = BF16 if bridge_bf16 else F32R
    nc = bacc.Bacc(
        "TRN2",
        target_bir_lowering=False,
        debug=False,
        enable_asserts=True,
        num_devices=N_CORES,
    )

    # ---- DRAM I/O ----------------------------------------------------------
    xb_d = nc.dram_tensor("xb", [S, D], BF16 if bridge_bf16 else F32,
                          kind="ExternalInput")
    xbT_d = nc.dram_tensor("xbT", [D, S], F32, kind="ExternalInput")
    xqT_d = nc.dram_tensor("xqT", [H, D, 256], F32, kind="ExternalInput")
    xres_d = nc.dram_tensor("xres", [SQ, D], F32, kind="ExternalInput")
    wq_d = nc.dram_tensor("wq", [D, H * D], F32, kind="ExternalInput")
    wk_d = nc.dram_tensor("wk", [D, H * D], F32, kind="ExternalInput")
    wv_d = nc.dram_tensor("wv", [H * D, D], F32, kind="ExternalInput")
    gamma_d = nc.dram_tensor("gamma", [D], F32, kind="ExternalInput")
    beta_d = nc.dram_tensor("beta", [D], F32, kind="ExternalInput")
    out_d = nc.dram_tensor("out", [SQ, D], F32, kind="ExternalOutput")
    probs_d = nc.dram_tensor("probs", [H, SQ, S], F32, kind="ExternalOutput")

    with tile.TileContext(nc) as tc, ExitStack() as ctx:
        # ---- pools ---------------------------------------------------------
        singles = ctx.enter_context(tc.tile_pool(name="singles", bufs=1))
        xt_pool = ctx.enter_context(tc.tile_pool(name="xt", bufs=2))
        kt_pool = ctx.enter_context(tc.tile_pool(name="kt", bufs=cfg["kt_bufs"]))
        es_pool = ctx.enter_context(tc.tile_pool(name="es", bufs=2))
        pr_pool = ctx.enter_context(tc.tile_pool(name="pr", bufs=2))
        est_pool = ctx.enter_context(tc.tile_pool(name="est", bufs=2))
        sm_pool = ctx.enter_context(tc.tile_pool(name="sm", bufs=4))
        o_pool = ctx.enter_context(tc.tile_pool(name="o", bufs=3))
        ln_pool = ctx.enter_context(tc.tile_pool(name="ln", bufs=4))

        # PSUM pools (8 banks of [128 x 2KB] total)
        sp_ps = ctx.enter_context(
            tc.tile_pool(name="sp", bufs=cfg["sp_bufs"], space="PSUM"))
        tp_ps = ctx.enter_context(
            tc.tile_pool(name="tp", bufs=cfg["tp_bufs"], space="PSUM"))
        at_ps = ctx.enter_context(
            tc.tile_pool(name="at", bufs=cfg["at_bufs"], space="PSUM"))
        pj_ps = ctx.enter_context(tc.tile_pool(name="pj", bufs=1, space="PSUM"))

        # ---- resident tiles ------------------------------------------------
        identity_f = singles.tile([128, 128], F32)
        make_identity(nc, identity_f[:])
        identity = singles.tile([128, 128], bridge_dt)
        nc.vector.tensor_copy(identity[:], identity_f[:])

        # x rows for PV: xb_sb[p, t, dd] = x[t*128+p, dd]
        xb_sb = singles.tile([128, 16, D], xb_dt)
        nc.sync.dma_start(
            xb_sb[:], xb_d.ap().rearrange("(t p) d -> p t d", p=128).bitcast(xb_dt)
        )

        # weights: w[c*128+p, n] -> [p, c, n]
        wq_sb = singles.tile([128, 2, H * D], F32R)
        nc.sync.dma_start(
            wq_sb[:], wq_d.ap().rearrange("(c p) n -> p c n", p=128).bitcast(F32R)
        )
        wk_sb = singles.tile([128, 2, H * D], F32R)
        nc.sync.dma_start(
            wk_sb[:], wk_d.ap().rearrange("(c p) n -> p c n", p=128).bitcast(F32R)
        )
        wv_sb = singles.tile([128, 8, D], F32R)
        nc.sync.dma_start(
            wv_sb[:], wv_d.ap().rearrange("(t p) d -> p t d", p=128).bitcast(F32R)
        )

        gamma_sb = singles.tile([128, D], F32)
        nc.gpsimd.dma_start(
            gamma_sb[:],
            bass.AP(tensor=gamma_d, offset=0, ap=[[0, 128], [1, D]]),
        )
        beta_sb = singles.tile([128, D], F32)
        nc.gpsimd.dma_start(
            beta_sb[:],
            bass.AP(tensor=beta_d, offset=0, ap=[[0, 128], [1, D]]),
        )
        eps_sb = singles.tile([128, 1], F32)
        nc.vector.memset(eps_sb[:], LN_EPS)

        # attT_sb[p, h, c2, i'] = att_h^T[c2*128+p, i']   (unnormalized-free:
        # probs used for PV are already normalized)
        attT_sb = singles.tile([128, H, 2, SQ], F32R)

        # ---- per-head attention -------------------------------------------
        for hh in range(H):
            # x^T columns for this head's K rows: xT_h[p, c, r] = x[h*512+r, c*128+p]
            xT_h = xt_pool.tile([128, 2, 512], F32R, tag="xT")
            nc.sync.dma_start(
                xT_h[:],
                xbT_d.ap()[:, hh * 512:(hh + 1) * 512].rearrange(
                    "(c p) j -> p c j", p=128
                ).bitcast(F32R),
            )
            # x^T columns for this head's Q rows
            xqT_h = xt_pool.tile([128, 2, 256], F32R, tag="xqT")
            nc.sync.dma_start(
                xqT_h[:],
                xqT_d.ap()[hh].rearrange("(c p) r -> p c r", p=128).bitcast(F32R),
            )

            # K^T_h[dd, j]: KT[p, c2, r, a] = K^T[c2*128+p, 4r+a]
            KT = kt_pool.tile([128, 2, 512, 4], F32R, tag="KT")
            QT = kt_pool.tile([128, 2, 256, 4], F32R, tag="QT")
            for a in range(4):
                for c2 in range(2):
                    pk = sp_ps.tile([128, 512], F32, tag="sps")
                    for c in range(2):
                        nc.tensor.matmul(
                            pk[:],
                            wk_sb[:, c, a * 256 + c2 * 128:a * 256 + c2 * 128 + 128],
                            xT_h[:, c, :],
                            start=(c == 0),
                            stop=(c == 1),
                        )
                    nc.vector.tensor_copy(KT[:, c2, :, a], pk[:])

                    pq = sp_ps.tile([128, 512], F32, tag="sps")
                    for c in range(2):
                        nc.tensor.matmul(
                            pq[:, 0:256],
                            wq_sb[:, c, a * 256 + c2 * 128:a * 256 + c2 * 128 + 128],
                            xqT_h[:, c, :],
                            start=(c == 0),
                            stop=(c == 1),
                        )
                    nc.vector.tensor_copy(QT[:, c2, :, a], pq[:, 0:256])

            # i'-chunks, processed in groups (PV free dim = icg_n*128)
            for icg in range(N_IC // icg_n):
                expST = est_pool.tile([128, 16, icg_n, 128], bridge_dt, tag="expST")
                for ic2 in range(icg_n):
                    ic = icg * icg_n + ic2
                    # scores S[i', j] for i' chunk, all j, accumulated over dd
                    expS = es_pool.tile([128, S], F32, tag="expS")
                    sums4 = sm_pool.tile([128, 4], F32, tag="sums4")
                    for jc in range(4):
                        ps = sp_ps.tile([128, 512], F32, tag="sps")
                        for c2 in range(2):
                            nc.tensor.matmul(
                                ps[:],
                                QT[:, c2, ic * 32:(ic + 1) * 32, :],
                                KT[:, c2, jc * 128:(jc + 1) * 128, :],
                                start=(c2 == 0),
                                stop=(c2 == 1),
                            )
                        # exp(S/16), with per-row running sum
                        nc.scalar.activation(
                            out=expS[:, jc * 512:(jc + 1) * 512],
                            in_=ps[:],
                            func=mybir.ActivationFunctionType.Exp,
                            scale=INV_SQRT_D,
                            accum_out=sums4[:, jc:jc + 1],
                        )
                    sums1 = sm_pool.tile([128, 1], F32, tag="sums1")
                    nc.vector.tensor_reduce(
                        out=sums1[:],
                        in_=sums4[:],
                        axis=mybir.AxisListType.X,
                        op=mybir.AluOpType.add,
                    )
                    recip = sm_pool.tile([128, 1], F32, tag="recip")
                    nc.vector.reciprocal(recip[:], sums1[:])

                    # normalized probs (fp32) -> HBM
                    probs_t = pr_pool.tile([128, S], F32R, tag="probs")
                    nc.vector.tensor_scalar_mul(probs_t[:], expS[:], recip[:])
                    nc.sync.dma_start(
                        probs_d.ap()[hh, ic * 128:(ic + 1) * 128, :],
                        probs_t[:].bitcast(F32),
                    )

                    # bridge: transpose probs to [j, i'] layout for PV
                    if bridge_bf16:
                        trans_src = pr_pool.tile([128, S], BF16, tag="probs16")
                        nc.gpsimd.tensor_copy(trans_src[:], probs_t[:])
                    else:
                        trans_src = probs_t
                    for jg in range(4):
                        tp = tp_ps.tile([128, 4, 128], bridge_dt, tag="tp")
                        for t4 in range(4):
                            jc2 = jg * 4 + t4
                            nc.tensor.transpose(
                                tp[:, t4, :],
                                trans_src[:, jc2 * 128:(jc2 + 1) * 128],
                                identity[:],
                            )
                        if jg % 2 == 0:
                            nc.scalar.copy(
                                expST[:, jg * 4:(jg + 1) * 4, ic2, :], tp[:]
                            )
                        else:
                            nc.vector.tensor_copy(
                                expST[:, jg * 4:(jg + 1) * 4, ic2, :], tp[:]
                            )

                # PV: attT[dd, i'] = sum_j x[j, dd] * P[i', j]
                nw = icg_n * 128
                for c2 in range(2):
                    pa = at_ps.tile([128, nw], F32, tag="at")
                    for jc2 in range(16):
                        nc.tensor.matmul(
                            pa[:],
                            xb_sb[:, jc2, c2 * 128:(c2 + 1) * 128],
                            expST[:, jc2, :, :],
                            start=(jc2 == 0),
                            stop=(jc2 == 15),
                        )
                    nc.vector.tensor_copy(
                        attT_sb[:, hh, c2, icg * nw:(icg + 1) * nw], pa[:]
                    )

        # ---- output projection + residual + layernorm ---------------------
        for ic in range(N_IC):
            pj = pj_ps.tile([128, 256], F32, tag="pj")
            k = 0
            for hh in range(H):
                for c2 in range(2):
                    nc.tensor.matmul(
                        pj[:],
                        attT_sb[:, hh, c2, ic * 128:(ic + 1) * 128],
                        wv_sb[:, hh * 2 + c2, :],
                        start=(k == 0),
                        stop=(k == 7),
                    )
                    k += 1
            xres_t = o_pool.tile([128, D], F32, tag="xres")
            nc.sync.dma_start(xres_t[:], xres_d.ap()[ic * 128:(ic + 1) * 128, :])
            res = o_pool.tile([128, D], F32, tag="res")
            nc.vector.tensor_add(res[:], pj[:], xres_t[:])

            st = ln_pool.tile([128, 6], F32, tag="st")
            nc.vector.bn_stats(st[:], res[:])
            mv = ln_pool.tile([128, 2], F32, tag="mv")
            nc.vector.bn_aggr(mv[:], st[:])
            sq = ln_pool.tile([128, 1], F32, tag="sq")
            nc.scalar.activation(
                out=sq[:],
                in_=mv[:, 1:2],
                func=mybir.ActivationFunctionType.Sqrt,
                bias=eps_sb[:],
            )
            rstd = ln_pool.tile([128, 1], F32, tag="rstd")
            nc.vector.reciprocal(rstd[:], sq[:])

            t1 = o_pool.tile([128, D], F32, tag="t1")
            nc.vector.tensor_scalar(
                out=t1[:],
                in0=res[:],
                scalar1=mv[:, 0:1],
                scalar2=rstd[:],
                op0=mybir.AluOpType.subtract,
                op1=mybir.AluOpType.mult,
            )
            t2 = o_pool.tile([128, D], F32, tag="t2")
            nc.vector.tensor_mul(t2[:], t1[:], gamma_sb[:])
            t3 = o_pool.tile([128, D], F32, tag="t3")
            nc.vector.tensor_add(t3[:], t2[:], beta_sb[:])
            nc.sync.dma_start(out_d.ap()[ic * 128:(ic + 1) * 128, :], t3[:])

    nc.compile()
    return nc


_NC = None
LAST_RESULTS = None


def _get_nc():
    global _NC
    if _NC is None:
        _NC = build_nc()
    return _NC


def make_in_maps(x, Wq, Wk, Wv, gamma, beta):
    x = np.ascontiguousarray(x, dtype=np.float32)
    Wq = np.ascontiguousarray(Wq, dtype=np.float32)
    Wk = np.ascontiguousarray(Wk, dtype=np.float32)
    Wv = np.ascontiguousarray(Wv, dtype=np.float32)
    gamma = np.ascontiguousarray(gamma, dtype=np.float32)
    beta = np.ascontiguousarray(beta, dtype=np.float32)
    in_maps = []
    for c in range(N_CORES):
        b, qh = divmod(c, 2)
        qoff = qh * SQ
        qoff4 = qoff // 4
        xb = np.ascontiguousarray(x[b])
        xbT = np.ascontiguousarray(xb.T)
        xqT = np.stack(
            [
                np.ascontiguousarray(xb[hh * 512 + qoff4:hh * 512 + qoff4 + 256].T)
                for hh in range(H)
            ]
        )
        xres = np.ascontiguousarray(xb[qoff:qoff + SQ])
        in_maps.append(
            {
                "xb": xb,
                "xbT": xbT,
                "xqT": xqT,
                "xres": xres,
                "wq": Wq,
                "wk": Wk,
                "wv": Wv,
                "gamma": gamma,
                "beta": beta,
            }
        )
    return in_maps


def kernel(x, Wq, Wk, Wv, gamma, beta):
    global LAST_RESULTS
    nc = _get_nc()
    in_maps = make_in_maps(x, Wq, Wk, Wv, gamma, beta)
    res = run_bass_kernel_spmd(
        nc,
        in_maps,
        core_ids=list(range(N_CORES)),
        trace=bool(int(os.environ.get("KERNEL_TRACE", "0"))),
    )
    LAST_RESULTS = res
    bs = x.shape[0]
    out = np.zeros((bs, S, D), np.float32)
    probs = np.zeros((bs, H, S, S), np.float32)
    for c in range(N_CORES):
        b, qh = divmod(c, 2)
        qoff = qh * SQ
        r = res.results[c]
        out[b, qoff:qoff + SQ] = r["out"]
        probs[b, :, qoff:qoff + SQ, :] = r["probs"]
    return out, probs
